# revision 27
# baseline (speedup 1.0000x reference)
"""EMA (first-order linear recurrence along T) for x[16, 512, 4096] f32.

y[..., 0] = x[..., 0];  y[..., t] = s_c * x[..., t] + (1 - s_c) * y[..., t-1]

Sharding: data-parallel over batch B across 8 cores (2 batches/core, each a
contiguous 16 MiB slab). Per core the (b, c) pairs form 1024 independent rows
of length T=4096; the recurrence maps 1:1 onto the TensorTensorScanArith
instruction (state = data0*state + data1 along the free dim, one recurrence
per partition).

Per 128-row block, all in-place on one SBUF tile X:
  DMA in (sync queue) -> ACT: X[:,1:] *= s (per-partition scale, scalar
  engine) -> scan on Vector: X = a*state + X with initial=0 (col 0 still
  holds raw x_0, so state_0 = x_0 exactly) -> DMA out (gpsimd queue, so a
  blocked out never head-of-line-blocks the remaining in-DMAs on sync).

The kernel is wire-bound (~90 us for 33.5 MB/core at the measured
~373 GB/s) and the Vector scan chain (8 x 8.66 us) is co-critical, so the
first and last blocks are split into half-T pieces to shorten pipeline fill
and drain, the weights DMA issues from the Vector queue to keep the sync
ring free for x, and a dummy activation hoists the one-time ACT_TABLE_LOAD
into the engine preamble.
"""

import numpy as np

import concourse.bacc as bacc
import concourse.mybir as mybir
import concourse.tile as tile
from concourse.bass_utils import run_bass_kernel_spmd

B, C, T = 16, 512, 4096
N_CORES = 8
B_PER = B // N_CORES          # 2 batches per core
ROWS = B_PER * C              # 1024 (b, c) rows per core
P = 128                       # SBUF partitions
N_BLOCKS = ROWS // P          # 8 row blocks per core
C_BLOCKS = C // P             # 4 channel blocks (weights layout)

DT = mybir.dt.float32
OP = mybir.AluOpType
ACT_COPY = mybir.ActivationFunctionType.Copy


def build(b_per=B_PER, c=C, t=T):
    rows = b_per * c
    n_blocks = rows // P
    c_blocks = c // P
    th = t // 2

    nc = bacc.Bacc("TRN2", target_bir_lowering=False, debug=False)

    x_in = nc.dram_tensor("x", [b_per, c, t], DT, kind="ExternalInput")
    w_in = nc.dram_tensor("weights", [c], DT, kind="ExternalInput")
    y_out = nc.dram_tensor("out", [b_per, c, t], DT, kind="ExternalOutput")

    xr = x_in.ap().rearrange("b c t -> (b c) t")   # [rows, t]
    yr = y_out.ap().rearrange("b c t -> (b c) t")
    # w4[p, j] = weights[j*128 + p] — column j holds channel block j
    wr = w_in.ap().rearrange("(j p) -> p j", p=P)  # [128, c_blocks]

    with tile.TileContext(nc) as tc:
        with (
            tc.tile_pool(name="const", bufs=1) as cpool,
            tc.tile_pool(name="xp", bufs=6) as xpool,
            tc.tile_pool(name="xh", bufs=4) as hpool,
        ):
            w4 = cpool.tile([P, c_blocks], DT)
            s4 = cpool.tile([P, c_blocks], DT)
            a4 = cpool.tile([P, c_blocks], DT)

            nc.sync.dma_start(w4[:], wr)
            # s = clamp(w, 0, 1); a = 1 - s  (gpsimd: keeps Vector scan-only)
            nc.gpsimd.tensor_scalar(s4[:], w4[:], 0.0, 1.0, OP.max, OP.min)
            nc.gpsimd.tensor_scalar(a4[:], s4[:], -1.0, 1.0, OP.mult, OP.add)

            def premul_scan(xt, lo, hi, j, first, init, premul_eng=None):
                # Premultiply s*x in place; for the row start skip col 0 so
                # the scan's first step gives state_0 = a*0 + x_0 exactly.
                a, b = lo + (1 if first else 0), hi
                if premul_eng is nc.gpsimd:
                    # Very first piece only: gpsimd is ready ~7 us before the
                    # scalar engine clears its preamble + ACT table load.
                    nc.gpsimd.tensor_scalar(
                        xt[:, a:b], xt[:, a:b], s4[:, j:j + 1], None, OP.mult)
                else:
                    nc.scalar.activation(
                        xt[:, a:b], xt[:, a:b], ACT_COPY, scale=s4[:, j:j + 1])
                nc.vector.tensor_tensor_scan(
                    xt[:, lo:hi],
                    a4[:, j:j + 1].to_broadcast((P, hi - lo)),
                    xt[:, lo:hi],
                    init,
                    OP.mult,
                    OP.add,
                )

            split_blocks = (0, n_blocks - 1)
            for k in range(n_blocks):
                j = k % c_blocks  # channel block of rows [k*128, (k+1)*128)
                r0 = k * P
                if k in split_blocks:
                    # Half-T pieces in separate tiles: shorter pipeline fill
                    # (k=0) and drain (last block) on the critical path.
                    xa = hpool.tile([P, th], DT)
                    xb = hpool.tile([P, th], DT)
                    nc.sync.dma_start(xa[:], xr[r0:r0 + P, 0:th])
                    nc.sync.dma_start(xb[:], xr[r0:r0 + P, th:t])
                    premul_scan(xa, 0, th, j, True, 0.0)
                    nc.gpsimd.dma_start(yr[r0:r0 + P, 0:th], xa[:])
                    premul_scan(xb, 0, th, j, False, xa[:, th - 1:th])
                    nc.gpsimd.dma_start(yr[r0:r0 + P, th:t], xb[:])
                else:
                    xt = xpool.tile([P, t], DT)
                    nc.sync.dma_start(xt[:], xr[r0:r0 + P, :])
                    premul_scan(xt, 0, t, j, True, 0.0)
                    nc.gpsimd.dma_start(yr[r0:r0 + P, :], xt[:])
    nc.compile()
    return nc


_NC_CACHE = []


def _enable_jax_compile_cache():
    # The NEFF compile (~1-4 min) rides jax's PJRT compile; a persistent
    # cache turns repeat fresh-process calls into ~20 s. Best-effort only.
    try:
        import jax
        jax.config.update("jax_compilation_cache_dir", "/tmp/jax_neff_cache")
        jax.config.update("jax_persistent_cache_min_compile_time_secs", 1.0)
    except Exception:
        pass


def kernel(x, weights, _run_kwargs=None):
    if not _NC_CACHE:
        _enable_jax_compile_cache()
        _NC_CACHE.append(build())
    nc = _NC_CACHE[0]
    x = np.ascontiguousarray(np.asarray(x, dtype=np.float32))
    weights = np.ascontiguousarray(np.asarray(weights, dtype=np.float32))
    in_maps = [
        {"x": x[i * B_PER:(i + 1) * B_PER], "weights": weights}
        for i in range(N_CORES)
    ]
    res = run_bass_kernel_spmd(
        nc, in_maps, core_ids=list(range(N_CORES)), **(_run_kwargs or {})
    )
    out = np.concatenate([res.results[i]["out"] for i in range(N_CORES)], axis=0)
    if _run_kwargs:
        kernel.last_results = res
    return out


# revision 29
# speedup vs baseline: 1.0073x; 1.0073x over previous
"""EMA (first-order linear recurrence along T) for x[16, 512, 4096] f32.

y[..., 0] = x[..., 0];  y[..., t] = s_c * x[..., t] + (1 - s_c) * y[..., t-1]

Sharding: data-parallel over batch B across 8 cores (2 batches/core, each a
contiguous 16 MiB slab). Per core the (b, c) pairs form 1024 independent rows
of length T=4096; the recurrence maps 1:1 onto the TensorTensorScanArith
instruction (state = data0*state + data1 along the free dim, one recurrence
per partition).

Per 128-row block, all in-place on one SBUF tile X:
  DMA in (sync queue) -> ACT: X[:,1:] *= s (per-partition scale, scalar
  engine) -> scan on Vector: X = a*state + X with initial=0 (col 0 still
  holds raw x_0, so state_0 = x_0 exactly) -> DMA out (gpsimd queue, so a
  blocked out never head-of-line-blocks the remaining in-DMAs on sync).

The kernel is wire-bound (~90 us for 33.5 MB/core at the measured
~373 GB/s) and the Vector scan chain (8 x 8.66 us) is co-critical, so the
first and last blocks are split into half-T pieces to shorten pipeline fill
and drain, the weights DMA issues from the Vector queue to keep the sync
ring free for x, and a dummy activation hoists the one-time ACT_TABLE_LOAD
into the engine preamble.
"""

import numpy as np

import concourse.bacc as bacc
import concourse.mybir as mybir
import concourse.tile as tile
from concourse.bass_utils import run_bass_kernel_spmd

B, C, T = 16, 512, 4096
N_CORES = 8
B_PER = B // N_CORES          # 2 batches per core
ROWS = B_PER * C              # 1024 (b, c) rows per core
P = 128                       # SBUF partitions
N_BLOCKS = ROWS // P          # 8 row blocks per core
C_BLOCKS = C // P             # 4 channel blocks (weights layout)

DT = mybir.dt.float32
OP = mybir.AluOpType
ACT_COPY = mybir.ActivationFunctionType.Copy


def build(b_per=B_PER, c=C, t=T):
    rows = b_per * c
    n_blocks = rows // P
    c_blocks = c // P
    th = t // 2

    nc = bacc.Bacc("TRN2", target_bir_lowering=False, debug=False)

    x_in = nc.dram_tensor("x", [b_per, c, t], DT, kind="ExternalInput")
    w_in = nc.dram_tensor("weights", [c], DT, kind="ExternalInput")
    y_out = nc.dram_tensor("out", [b_per, c, t], DT, kind="ExternalOutput")

    xr = x_in.ap().rearrange("b c t -> (b c) t")   # [rows, t]
    yr = y_out.ap().rearrange("b c t -> (b c) t")
    # w4[p, j] = weights[j*128 + p] — column j holds channel block j
    wr = w_in.ap().rearrange("(j p) -> p j", p=P)  # [128, c_blocks]

    with tile.TileContext(nc) as tc:
        with (
            tc.tile_pool(name="const", bufs=1) as cpool,
            tc.tile_pool(name="xp", bufs=6) as xpool,
            tc.tile_pool(name="xh", bufs=4) as hpool,
        ):
            w4 = cpool.tile([P, c_blocks], DT)
            s4 = cpool.tile([P, c_blocks], DT)
            a4 = cpool.tile([P, c_blocks], DT)

            nc.sync.dma_start(w4[:], wr)
            # s = clamp(w, 0, 1); a = 1 - s  (gpsimd: keeps Vector scan-only)
            nc.gpsimd.tensor_scalar(s4[:], w4[:], 0.0, 1.0, OP.max, OP.min)
            nc.gpsimd.tensor_scalar(a4[:], s4[:], -1.0, 1.0, OP.mult, OP.add)

            def premul_scan(xt, lo, hi, j, first, init):
                # Premultiply s*x in place; for the row start skip col 0 so
                # the scan's first step gives state_0 = a*0 + x_0 exactly.
                a, b = lo + (1 if first else 0), hi
                nc.scalar.activation(
                    xt[:, a:b], xt[:, a:b], ACT_COPY, scale=s4[:, j:j + 1])
                nc.vector.tensor_tensor_scan(
                    xt[:, lo:hi],
                    a4[:, j:j + 1].to_broadcast((P, hi - lo)),
                    xt[:, lo:hi],
                    init,
                    OP.mult,
                    OP.add,
                )

            split_blocks = (0, n_blocks - 1)
            outs = []  # (dram_dst, tile_src) — emitted after all in-DMAs
            for k in range(n_blocks):
                j = k % c_blocks  # channel block of rows [k*128, (k+1)*128)
                r0 = k * P
                if k in split_blocks:
                    # Half-T pieces in separate tiles: shorter pipeline fill
                    # (k=0) and drain (last block) on the critical path.
                    xa = hpool.tile([P, th], DT)
                    xb = hpool.tile([P, th], DT)
                    nc.sync.dma_start(xa[:], xr[r0:r0 + P, 0:th])
                    nc.sync.dma_start(xb[:], xr[r0:r0 + P, th:t])
                    premul_scan(xa, 0, th, j, True, 0.0)
                    outs.append((yr[r0:r0 + P, 0:th], xa[:]))
                    premul_scan(xb, 0, th, j, False, xa[:, th - 1:th])
                    outs.append((yr[r0:r0 + P, th:t], xb[:]))
                else:
                    xt = xpool.tile([P, t], DT)
                    nc.sync.dma_start(xt[:], xr[r0:r0 + P, :])
                    premul_scan(xt, 0, t, j, True, 0.0)
                    outs.append((yr[r0:r0 + P, :], xt[:]))
            # Out-DMAs on the sync HWDGE ring, all emitted after the ins:
            # every in-DMA has already issued before the first out's wait
            # can block the ring, and HWDGE writes fan out across all 16
            # SDMA slots (SWDGE has a single context).
            for dst, src in outs:
                nc.sync.dma_start(dst, src)
    nc.compile()
    return nc


_NC_CACHE = []


def _enable_jax_compile_cache():
    # The NEFF compile (~1-4 min) rides jax's PJRT compile; a persistent
    # cache turns repeat fresh-process calls into ~20 s. Best-effort only.
    try:
        import jax
        jax.config.update("jax_compilation_cache_dir", "/tmp/jax_neff_cache")
        jax.config.update("jax_persistent_cache_min_compile_time_secs", 1.0)
    except Exception:
        pass


def kernel(x, weights, _run_kwargs=None):
    if not _NC_CACHE:
        _enable_jax_compile_cache()
        _NC_CACHE.append(build())
    nc = _NC_CACHE[0]
    x = np.ascontiguousarray(np.asarray(x, dtype=np.float32))
    weights = np.ascontiguousarray(np.asarray(weights, dtype=np.float32))
    in_maps = [
        {"x": x[i * B_PER:(i + 1) * B_PER], "weights": weights}
        for i in range(N_CORES)
    ]
    res = run_bass_kernel_spmd(
        nc, in_maps, core_ids=list(range(N_CORES)), **(_run_kwargs or {})
    )
    out = np.concatenate([res.results[i]["out"] for i in range(N_CORES)], axis=0)
    if _run_kwargs:
        kernel.last_results = res
    return out


# revision 35
# speedup vs baseline: 1.0088x; 1.0015x over previous
"""EMA (first-order linear recurrence along T) for x[16, 512, 4096] f32.

y[..., 0] = x[..., 0];  y[..., t] = s_c * x[..., t] + (1 - s_c) * y[..., t-1]

Sharding: data-parallel over batch B across 8 cores (2 batches/core, each a
contiguous 16 MiB slab). Per core the (b, c) pairs form 1024 independent rows
of length T=4096; the recurrence maps 1:1 onto the TensorTensorScanArith
instruction (state = data0*state + data1 along the free dim, one recurrence
per partition).

Per 128-row block, all in-place on one SBUF tile X:
  DMA in -> ACT: X[:,1:] *= s (per-partition scale, scalar engine) -> scan
  on Vector: X = a*state + X with initial=0 (col 0 still holds raw x_0, so
  state_0 = x_0 exactly) -> DMA out.

All DMAs ride the sync HWDGE ring (fans each transfer across all 16 SDMA
slots; SWDGE has a single context and writes ~25% slower), with every out
emitted after every in so an out's scan-wait can never head-of-line-block
an input load. The kernel is wire-bound: ~85.6 us for 33.5 MB/core at the
measured ~392 GB/s sustained, plus ~6.9 us fixed Tile preamble. The first
and last blocks are split into half-T pieces so pipeline fill and drain
stay off the critical path. The Tile schedule is sharply sensitive: moving
the weights DMA, adding preamble ops, or re-homing the premultiply all
regressed 2-26 us — change emission order only with bench evidence.
"""

import numpy as np

import concourse.bacc as bacc
import concourse.mybir as mybir
import concourse.tile as tile
from concourse.bass_utils import run_bass_kernel_spmd

B, C, T = 16, 512, 4096
N_CORES = 8
B_PER = B // N_CORES          # 2 batches per core
ROWS = B_PER * C              # 1024 (b, c) rows per core
P = 128                       # SBUF partitions
N_BLOCKS = ROWS // P          # 8 row blocks per core
C_BLOCKS = C // P             # 4 channel blocks (weights layout)

DT = mybir.dt.float32
OP = mybir.AluOpType
ACT_COPY = mybir.ActivationFunctionType.Copy


def build(b_per=B_PER, c=C, t=T):
    rows = b_per * c
    n_blocks = rows // P
    c_blocks = c // P
    th = t // 2

    nc = bacc.Bacc("TRN2", target_bir_lowering=False, debug=False)

    x_in = nc.dram_tensor("x", [b_per, c, t], DT, kind="ExternalInput")
    w_in = nc.dram_tensor("weights", [c], DT, kind="ExternalInput")
    y_out = nc.dram_tensor("out", [b_per, c, t], DT, kind="ExternalOutput")

    xr = x_in.ap().rearrange("b c t -> (b c) t")   # [rows, t]
    yr = y_out.ap().rearrange("b c t -> (b c) t")
    # w4[p, j] = weights[j*128 + p] — column j holds channel block j
    wr = w_in.ap().rearrange("(j p) -> p j", p=P)  # [128, c_blocks]

    with tile.TileContext(nc) as tc:
        with (
            tc.tile_pool(name="const", bufs=1) as cpool,
            tc.tile_pool(name="xp", bufs=6) as xpool,
            tc.tile_pool(name="xh", bufs=4) as hpool,
        ):
            w4 = cpool.tile([P, c_blocks], DT)
            s4 = cpool.tile([P, c_blocks], DT)
            a4 = cpool.tile([P, c_blocks], DT)

            nc.sync.dma_start(w4[:], wr)
            # s = clamp(w, 0, 1); a = 1 - s  (gpsimd: keeps Vector scan-only)
            nc.gpsimd.tensor_scalar(s4[:], w4[:], 0.0, 1.0, OP.max, OP.min)
            nc.gpsimd.tensor_scalar(a4[:], s4[:], -1.0, 1.0, OP.mult, OP.add)

            def premul_scan(xt, lo, hi, j, first, init):
                # Premultiply s*x in place; for the row start skip col 0 so
                # the scan's first step gives state_0 = a*0 + x_0 exactly.
                a, b = lo + (1 if first else 0), hi
                nc.scalar.activation(
                    xt[:, a:b], xt[:, a:b], ACT_COPY, scale=s4[:, j:j + 1])
                nc.vector.tensor_tensor_scan(
                    xt[:, lo:hi],
                    a4[:, j:j + 1].to_broadcast((P, hi - lo)),
                    xt[:, lo:hi],
                    init,
                    OP.mult,
                    OP.add,
                )

            split_blocks = (0, n_blocks - 1)
            outs = []  # (dram_dst, tile_src) — emitted after all in-DMAs
            for k in range(n_blocks):
                j = k % c_blocks  # channel block of rows [k*128, (k+1)*128)
                r0 = k * P
                if k in split_blocks:
                    # Half-T pieces in separate tiles: shorter pipeline fill
                    # (k=0) and drain (last block) on the critical path.
                    xa = hpool.tile([P, th], DT)
                    xb = hpool.tile([P, th], DT)
                    nc.sync.dma_start(xa[:], xr[r0:r0 + P, 0:th])
                    nc.sync.dma_start(xb[:], xr[r0:r0 + P, th:t])
                    premul_scan(xa, 0, th, j, True, 0.0)
                    outs.append((yr[r0:r0 + P, 0:th], xa[:]))
                    premul_scan(xb, 0, th, j, False, xa[:, th - 1:th])
                    outs.append((yr[r0:r0 + P, th:t], xb[:]))
                else:
                    xt = xpool.tile([P, t], DT)
                    nc.sync.dma_start(xt[:], xr[r0:r0 + P, :])
                    premul_scan(xt, 0, t, j, True, 0.0)
                    outs.append((yr[r0:r0 + P, :], xt[:]))
            # Out-DMAs on the sync HWDGE ring, all emitted after the ins:
            # every in-DMA has already issued before the first out's wait
            # can block the ring, and HWDGE writes fan out across all 16
            # SDMA slots (SWDGE has a single context).
            for dst, src in outs:
                nc.sync.dma_start(dst, src)
    nc.compile()
    return nc


_NC_CACHE = []


def _enable_jax_compile_cache():
    # The NEFF compile (~1-4 min) rides jax's PJRT compile; a persistent
    # cache turns repeat fresh-process calls into ~20 s. Best-effort only.
    try:
        import jax
        jax.config.update("jax_compilation_cache_dir", "/tmp/jax_neff_cache")
        jax.config.update("jax_persistent_cache_min_compile_time_secs", 1.0)
    except Exception:
        pass


def kernel(x, weights, _run_kwargs=None):
    if not _NC_CACHE:
        _enable_jax_compile_cache()
        _NC_CACHE.append(build())
    nc = _NC_CACHE[0]
    x = np.ascontiguousarray(np.asarray(x, dtype=np.float32))
    weights = np.ascontiguousarray(np.asarray(weights, dtype=np.float32))
    in_maps = [
        {"x": x[i * B_PER:(i + 1) * B_PER], "weights": weights}
        for i in range(N_CORES)
    ]
    res = run_bass_kernel_spmd(
        nc, in_maps, core_ids=list(range(N_CORES)), **(_run_kwargs or {})
    )
    out = np.concatenate([res.results[i]["out"] for i in range(N_CORES)], axis=0)
    if _run_kwargs:
        kernel.last_results = res
    return out
